# revision 1
# baseline (speedup 1.0000x reference)
"""Trainium2 Bass kernel for nn_DownConvLayers (5-layer GCN, N=100k nodes, E=1.6M edges).

Strategy (8 NeuronCores, SPMD):
  - Shard destination nodes across the 8 cores (12500 nodes/core, padded to
    98 tiles x 128 slots, load-balanced by in-degree).
  - GCN norm factorizes: norm_e = dinv[src] * dinv[dst].  Fold dinv[src] into
    the per-node feature rows H (scaled after the X@W matmul) and dinv[dst]
    into the aggregation epilogue — no per-edge scaling needed.
  - Per layer: H_own = act @ W (PE), scale rows by dinv, AllGather H across
    cores, then per dst-tile: bulk-gather all message rows via dma_gather
    (the Q7 MoE gather path; 4 SWDGE queues in parallel, H table split in 4
    int16-addressable chunks), build a 0/1 scatter matrix S on DVE
    (slot==iota compare), and segment-sum via PE matmuls
    psum[128 dst, F] += S_b^T @ M_b accumulated over blocks.
  - Layers with F<64 gather from a 64-col padded copy of the table (dma_gather
    needs 256B-multiple element size/stride); the pad columns are never read.
  - Epilogue per tile: scale by dinv[dst], transpose (PE), relu(x + b) on ACT
    into the next layer's activation buffer (kept transposed in SBUF so the
    next W-matmul needs no transpose).
  - Final layer writes relu(dinv*agg + b5) rows straight to the output.

Host side: preprocessing (degree, tile assignment, edge->block packing,
permutations) in numpy; output unpermuted and assembled from the 8 shards.
"""

import heapq
import math

import ml_dtypes
import numpy as np

import concourse.bacc as bacc
import concourse.tile as tile
from concourse import bass, mybir
from concourse.bass_utils import run_bass_kernel_spmd
from concourse.masks import make_identity

N_CORES = 8
IN_C = 128
WIDTHS = [128, 64, 32, 16, 8]
F32 = mybir.dt.float32
BF16 = mybir.dt.bfloat16
I16 = mybir.dt.int16
NQ = 4          # SWDGE queues
GMIN = 64       # min gather row width (256B f32)
SUBB = 5        # max blocks (x128 idxs) per dma_gather op
NL = 5          # layers to build (debug knob)


# ---------------------------------------------------------------- device code


def _build_program(T, B_c, C, chunk_rows, per_pad):
    """T tiles/core, C chunks, B_c blocks per (tile, chunk)."""
    HN = N_CORES * per_pad
    B = C * B_c                  # blocks per tile
    S16 = B_c * 8                # idx cols per chunk (wrapped 16, int16)
    fan_in = [IN_C] + WIDTHS[:-1]
    nc = bacc.Bacc("TRN2", target_bir_lowering=False, debug=False,
                   num_devices=N_CORES, num_swdge_queues=NQ)

    xT_in = nc.dram_tensor("xT", [128, per_pad], F32, kind="ExternalInput")
    idx_in = nc.dram_tensor("idx", [T, 128, C * S16], I16,
                            kind="ExternalInput")
    slot_in = nc.dram_tensor("slot", [T, 128, B], BF16, kind="ExternalInput")
    dinv_in = nc.dram_tensor("dinv", [128, T], F32, kind="ExternalInput")
    iota_in = nc.dram_tensor("iota", [128, 128], BF16, kind="ExternalInput")
    W_in = [
        nc.dram_tensor(f"W{i + 1}", [fi, fo], F32, kind="ExternalInput")
        for i, (fi, fo) in enumerate(zip(fan_in, WIDTHS))
    ]
    b_in = [
        nc.dram_tensor(f"b{i + 1}", [WIDTHS[i]], F32, kind="ExternalInput")
        for i in range(max(NL - 1, 0))
    ]
    b5b_in = nc.dram_tensor("b5b", [128, WIDTHS[NL - 1]], F32, kind="ExternalInput")
    out_ext = nc.dram_tensor("out", [per_pad, WIDTHS[NL - 1]], F32,
                             kind="ExternalOutput")

    with tile.TileContext(nc) as tc:
        with (
            tc.tile_pool(name="dram", bufs=1, space="DRAM") as dram,
            tc.tile_pool(name="consts", bufs=1) as consts,
            tc.tile_pool(name="acts", bufs=2) as acts,
            tc.tile_pool(name="mpool", bufs=4) as mpool,
            tc.tile_pool(name="spool", bufs=3) as spool,
            tc.tile_pool(name="small", bufs=6) as small,
            tc.tile_pool(name="hsb", bufs=6) as hsb,
            tc.tile_pool(name="psA", bufs=4, space="PSUM") as psA,
            tc.tile_pool(name="psT", bufs=2, space="PSUM") as psT,
            tc.tile_pool(name="psW", bufs=2, space="PSUM") as psW,
        ):
            ident = consts.tile([128, 128], F32)
            make_identity(nc, ident[:])
            iota = consts.tile([128, 128], BF16)
            nc.sync.dma_start(iota[:], iota_in[:])
            dinv = consts.tile([128, T], F32)
            nc.sync.dma_start(dinv[:], dinv_in[:])
            W_sb = []
            for i, (fi, fo) in enumerate(zip(fan_in, WIDTHS)):
                w = consts.tile([fi, fo], F32, tag=f"W{i}")
                nc.sync.dma_start(w[:], W_in[i][:])
                W_sb.append(w)
            b_sb = []
            for i in range(NL - 1):
                bb = consts.tile([WIDTHS[i], 1], F32, tag=f"b{i}")
                nc.sync.dma_start(bb[:], b_in[i][:, None])
                b_sb.append(bb)
            b5b = consts.tile([128, WIDTHS[NL - 1]], F32)
            nc.sync.dma_start(b5b[:], b5b_in[:])

            act = acts.tile([128, per_pad], F32, tag="act")
            nc.sync.dma_start(act[:], xT_in[:])

            for l, (fi, fo) in list(enumerate(zip(fan_in, WIDTHS)))[:NL]:
                g = 128  # bf16 table rows are 128 cols = 256B (dma_gather min)
                # ---- transform: H_own = (act.T @ W) * dinv[rows] ----
                h_own = dram.tile([per_pad, g], BF16, tag=f"hown{l}")
                h_gt = dram.tile([HN, g], BF16, tag=f"hfull{l}")
                for t0 in range(0, T, 4):
                    nt4 = min(4, T - t0)
                    hs = hsb.tile([128, 4 * fo], BF16, tag="hs")
                    for tt in range(nt4):
                        t = t0 + tt
                        hp = psW.tile([128, fo], F32, tag="psW", space="PSUM")
                        nc.tensor.matmul(
                            out=hp[:],
                            lhsT=act[:fi, t * 128:(t + 1) * 128],
                            rhs=W_sb[l][:],
                            start=True, stop=True,
                        )
                        nc.vector.tensor_scalar(
                            hs[:, tt * fo:(tt + 1) * fo], hp[:],
                            dinv[:, t:t + 1], None,
                            mybir.AluOpType.mult,
                        )
                    dst = (h_own[t0 * 128:(t0 + nt4) * 128, :]
                           .rearrange("(b p) f -> p b f", p=128))
                    if g > fo:
                        dst = dst[:, :, :fo]
                    nc.sync.dma_start(
                        dst,
                        hs[:, :nt4 * fo].rearrange("p (b f) -> p b f", b=nt4))

                nc.gpsimd.collective_compute(
                    "AllGather",
                    mybir.AluOpType.bypass,
                    replica_groups=[list(range(N_CORES))],
                    ins=[h_own.opt()],
                    outs=[h_gt.opt()],
                )

                # ---- aggregate per destination tile ----
                if l < NL - 1:
                    next_act = acts.tile([128, per_pad], F32, tag="act")
                else:
                    next_act = None
                ob = None
                for t in range(T):
                    idxt = small.tile([128, C * S16], I16, tag="idx")
                    nc.scalar.dma_start(idxt[:], idx_in[t])
                    slott = small.tile([128, B], BF16, tag="slot")
                    nc.scalar.dma_start(slott[:], slot_in[t])
                    S = spool.tile([128, B * 128], BF16, tag="S")
                    nc.vector.tensor_tensor(
                        out=S[:].rearrange("p (b j) -> p b j", b=B),
                        in0=slott[:, :, None].to_broadcast([128, B, 128]),
                        in1=iota[:, None, :].to_broadcast([128, B, 128]),
                        op=mybir.AluOpType.is_equal,
                    )
                    M = mpool.tile([128, B * g], BF16, tag="M")
                    qn = 0
                    for k in range(C):
                        k0 = k * chunk_rows
                        k1 = min(k0 + chunk_rows, HN)
                        for j in range(0, B_c, SUBB):
                            nb = min(SUBB, B_c - j)
                            b0 = k * B_c + j
                            nc.gpsimd.dma_gather(
                                out_ap=M[:, b0 * g:(b0 + nb) * g]
                                    .rearrange("p (c f) -> p c f", c=nb),
                                in_ap=h_gt[k0:k1, :],
                                idxs_ap=idxt[:, k * S16 + j * 8:
                                             k * S16 + (j + nb) * 8],
                                num_idxs=nb * 128,
                                num_idxs_reg=nb * 128,
                                elem_size=g,
                                elem_step=g,
                                single_packet=False,
                                queue_num=(t * C + qn) % NQ,
                            )
                            qn += 1
                    agg = psA.tile([128, fo], F32, tag="agg", space="PSUM")
                    for b in range(B):
                        nc.tensor.matmul(
                            out=agg[:],
                            lhsT=S[:, b * 128:(b + 1) * 128],
                            rhs=M[:, b * g:b * g + fo],
                            start=(b == 0), stop=(b == B - 1),
                        )
                    sc = hsb.tile([128, fo], F32, tag="sc")
                    nc.vector.tensor_scalar(
                        sc[:], agg[:], dinv[:, t:t + 1], None,
                        mybir.AluOpType.mult,
                    )
                    if l < NL - 1:
                        tp = psT.tile([128, 128], F32, tag="psT", space="PSUM")
                        nc.tensor.transpose(
                            out=tp[:fo, :], in_=sc[:], identity=ident[:])
                        nc.scalar.activation(
                            out=next_act[:fo, t * 128:(t + 1) * 128],
                            in_=tp[:fo, :],
                            func=mybir.ActivationFunctionType.Relu,
                            bias=b_sb[l][:],
                            scale=1.0,
                        )
                    else:
                        sc2 = hsb.tile([128, fo], F32, tag="sc2")
                        nc.vector.tensor_add(sc2[:], sc[:], b5b[:, :fo])
                        if t % 4 == 0:
                            ob = hsb.tile([128, 4 * fo], F32, tag="ob")
                        nc.scalar.activation(
                            out=ob[:, (t % 4) * fo:(t % 4 + 1) * fo],
                            in_=sc2[:],
                            func=mybir.ActivationFunctionType.Relu,
                        )
                        if t % 4 == 3 or t == T - 1:
                            t0 = (t // 4) * 4
                            nt4 = t - t0 + 1
                            nc.sync.dma_start(
                                out_ext[t0 * 128:(t0 + nt4) * 128, :]
                                    .rearrange("(b p) f -> p b f", p=128),
                                ob[:, :nt4 * fo]
                                    .rearrange("p (b f) -> p b f", b=nt4))
                act = next_act
    nc.compile()
    return nc


# ------------------------------------------------------------------ host prep


def _preprocess(x, edge_index):
    N = x.shape[0]
    assert N % N_CORES == 0
    per = N // N_CORES
    T = math.ceil(per / 128)
    per_pad = T * 128
    HN = N_CORES * per_pad
    C = max(1, math.ceil(HN / 32768))
    chunk_rows = math.ceil(HN / C)
    assert chunk_rows <= 32768

    src = np.concatenate([edge_index[0], np.arange(N)]).astype(np.int64)
    dst = np.concatenate([edge_index[1], np.arange(N)]).astype(np.int64)
    deg = np.bincount(dst, minlength=N).astype(np.float32)
    dinv = (1.0 / np.sqrt(deg)).astype(np.float32)

    node_core = (np.arange(N) // per).astype(np.int32)

    # per-core LPT assignment of nodes to tiles, balancing in-degree
    tile_of = np.empty(N, np.int32)
    slot_of = np.empty(N, np.int32)
    for c in range(N_CORES):
        nodes = np.arange(c * per, (c + 1) * per)
        d = deg[nodes]
        order = np.argsort(-d, kind="stable")
        heap = [(0.0, t) for t in range(T)]
        heapq.heapify(heap)
        counts = np.zeros(T, np.int32)
        tl = np.empty(per, np.int32)
        sl = np.empty(per, np.int32)
        for i in order:
            while True:
                load, t = heapq.heappop(heap)
                if counts[t] < 128:
                    break
            tl[i] = t
            sl[i] = counts[t]
            counts[t] += 1
            heapq.heappush(heap, (load + float(d[i]), t))
        tile_of[nodes] = tl
        slot_of[nodes] = sl

    tilepos = tile_of * 128 + slot_of
    hrow = node_core.astype(np.int64) * per_pad + tilepos  # row in H_full

    # pack edges into (core, tile, chunk, block, lane)
    e_core = node_core[dst]
    e_tile = tile_of[dst]
    e_chunk = (hrow[src] // chunk_rows).astype(np.int64)
    e_local = (hrow[src] % chunk_rows).astype(np.int64)
    key = (e_core.astype(np.int64) * T + e_tile) * C + e_chunk
    counts_e = np.bincount(key, minlength=N_CORES * T * C)
    B_c = int(math.ceil(counts_e.max() / 128))
    B = C * B_c
    eorder = np.argsort(key, kind="stable")
    starts = np.zeros(N_CORES * T * C + 1, np.int64)
    starts[1:] = np.cumsum(counts_e)
    pos = np.arange(len(key)) - starts[key[eorder]]

    so = eorder
    sc_, st_, sk_ = e_core[so], e_tile[so], e_chunk[so]

    # int16 wrapped indices: flat msg i of (tile,chunk) -> wrapped[p%16, i//16]
    S16 = B_c * 8
    idxw = np.zeros((N_CORES, T, C, 16, S16), np.int16)
    idxw[sc_, st_, sk_, pos % 16, pos // 16] = e_local[so].astype(np.int16)
    idx_arr = np.tile(idxw, (1, 1, 1, 8, 1))  # replicate to 128 partitions
    idx_arr = np.ascontiguousarray(
        idx_arr.transpose(0, 1, 3, 2, 4).reshape(N_CORES, T, 128, C * S16))

    slot_arr = np.full((N_CORES, T, 128, B), 255.0, ml_dtypes.bfloat16)
    slot_arr[sc_, st_, pos % 128, sk_ * B_c + pos // 128] = \
        slot_of[dst[so]].astype(ml_dtypes.bfloat16)

    # x transposed+permuted per core
    xt = np.zeros((N_CORES, per_pad, IN_C), np.float32)
    xt[node_core, tilepos] = x
    xt = np.ascontiguousarray(xt.transpose(0, 2, 1))

    dv = np.ones((N_CORES, T, 128), np.float32)
    dv[node_core, tile_of, slot_of] = dinv
    dv = np.ascontiguousarray(dv.transpose(0, 2, 1))

    return dict(
        N=N, per=per, T=T, B_c=B_c, C=C, chunk_rows=chunk_rows,
        per_pad=per_pad, idx=idx_arr, slot=slot_arr, xt=xt, dv=dv,
        node_core=node_core, tilepos=tilepos,
    )


_PROGRAM_CACHE = {}


def kernel(**inputs):
    x = np.ascontiguousarray(np.asarray(inputs["x"], dtype=np.float32))
    edge_index = np.asarray(inputs["edge_index"])
    Ws = [np.ascontiguousarray(np.asarray(inputs[f"W{i + 1}"], np.float32))
          for i in range(5)]
    bs = [np.ascontiguousarray(np.asarray(inputs[f"b{i + 1}"], np.float32))
          for i in range(5)]


    p = _preprocess(x, edge_index)

    key = (p["T"], p["B_c"], p["C"], p["chunk_rows"], p["per_pad"])
    if key not in _PROGRAM_CACHE:
        _PROGRAM_CACHE[key] = _build_program(*key)
    nc = _PROGRAM_CACHE[key]

    iota = np.ascontiguousarray(
        np.tile(np.arange(128, dtype=ml_dtypes.bfloat16), (128, 1)))
    b5b = np.ascontiguousarray(np.tile(bs[NL - 1], (128, 1)))

    in_maps = []
    for c in range(N_CORES):
        m = {
            "xT": p["xt"][c],
            "idx": p["idx"][c],
            "slot": p["slot"][c],
            "dinv": p["dv"][c],
            "iota": iota,
            "b5b": b5b,
        }
        for i in range(5):
            m[f"W{i + 1}"] = Ws[i]
        for i in range(NL - 1):
            m[f"b{i + 1}"] = bs[i]
        in_maps.append(m)

    res = run_bass_kernel_spmd(nc, in_maps, core_ids=list(range(N_CORES)))

    shards = np.stack([res.results[c]["out"] for c in range(N_CORES)])
    out = np.empty((p["N"], WIDTHS[NL - 1]), np.float32)
    out[:] = shards[p["node_core"], p["tilepos"]]
    return out



# revision 6
# speedup vs baseline: 1.4854x; 1.4854x over previous
"""Trainium2 Bass kernel for nn_DownConvLayers (5-layer GCN, N=100k, E=1.6M).

Strategy (8 NeuronCores, SPMD), v2:
  - Shard destination nodes across 8 cores; T=100 tiles x 128 slots per core
    (LPT-balanced by in-degree).  Norm factorizes: fold dinv[src] into the
    feature table rows, dinv[dst] into the epilogue.
  - Table rows are quarter-interleaved: chunk k holds quarter k of every
    core's rows, so the per-layer AllGather splits into 4 sub-collectives
    (each [3200 rows x 128 bf16] -> [25600 x 128]) that pipeline with the
    gathers of earlier chunks.  h_gt output is addr_space="Shared".
  - Gather ops are merged: ONE dma_gather per (5-tile group, chunk) with
    exact per-(tile,chunk) block counts (max across cores for SPMD shape
    uniformity) -> 80 ops/layer instead of 784, killing the ~1.8us/op SWDGE
    descriptor-generation fixed cost that dominated v1 (GpSimd 83% busy).
  - idx/slot arrays are layer-invariant: loaded into SBUF once.
  - Scatter matrices S are built just-in-time per (tile,chunk) on DVE;
    segment-sum psum[128 dst, fo] += S_b^T @ M_b on PE.
  - Activations/weights bf16 (halves SBUF + PE stream cost); epilogue:
    scale by dinv[dst], PE-transpose, relu(x+b) on ACT into next act.
"""

import heapq
import math

import ml_dtypes
import numpy as np

import concourse.bacc as bacc
import concourse.tile as tile
from concourse import bass, mybir
from concourse.bass_utils import run_bass_kernel_spmd
from concourse.masks import make_identity

N_CORES = 8
IN_C = 128
WIDTHS = [128, 64, 32, 16, 8]
F32 = mybir.dt.float32
BF16 = mybir.dt.bfloat16
I16 = mybir.dt.int16
NQ = 4          # SWDGE queues
NL = 5          # layers to build (debug knob)
T = 100         # tiles per core
TPQ = 25        # tiles per quarter
Q = 4           # quarters == chunks
G = 5           # tiles per gather group
NGRP = T // G


# ---------------------------------------------------------------- device code


def _build_program(struct):
    per_pad = struct["per_pad"]
    chunk_rows = struct["chunk_rows"]
    HN = struct["HN"]
    NB = struct["NB"]          # [NGRP][Q] blocks per op
    nb_u = struct["nb_u"]      # [T][Q] blocks per (tile, chunk)
    off_u = struct["off_u"]    # [T][Q] block offset of tile within its op
    col0 = struct["col0"]      # [NGRP][Q] idx col offset of op
    blk0 = struct["blk0"]      # [NGRP][Q] global block offset of op
    TOTCOL = struct["TOTCOL"]
    TOTBLK = struct["TOTBLK"]
    NBMAX = struct["NBMAX"]
    SMAX = struct["SMAX"]
    g_el = 128                 # gather element: 128 bf16 cols = 256B

    fan_in = [IN_C] + WIDTHS[:-1]
    nc = bacc.Bacc("TRN2", target_bir_lowering=False, debug=False,
                   num_devices=N_CORES, num_swdge_queues=NQ)

    xT_in = nc.dram_tensor("xT", [128, per_pad], BF16, kind="ExternalInput")
    idx_in = nc.dram_tensor("idx", [128, TOTCOL], I16, kind="ExternalInput")
    slot_in = nc.dram_tensor("slot", [128, TOTBLK], BF16,
                             kind="ExternalInput")
    dinv_in = nc.dram_tensor("dinv", [128, T], F32, kind="ExternalInput")
    iota_in = nc.dram_tensor("iota", [128, 128], BF16, kind="ExternalInput")
    W_in = [
        nc.dram_tensor(f"W{i + 1}", [fi, fo], BF16, kind="ExternalInput")
        for i, (fi, fo) in enumerate(zip(fan_in, WIDTHS))
    ]
    b_in = [
        nc.dram_tensor(f"b{i + 1}", [WIDTHS[i]], F32, kind="ExternalInput")
        for i in range(max(NL - 1, 0))
    ]
    b5b_in = nc.dram_tensor("b5b", [128, WIDTHS[NL - 1]], F32,
                            kind="ExternalInput")
    out_ext = nc.dram_tensor("out", [per_pad, WIDTHS[NL - 1]], F32,
                             kind="ExternalOutput")

    with tile.TileContext(nc) as tc:
        with (
            tc.tile_pool(name="dram", bufs=1, space="DRAM") as dram,
            tc.tile_pool(name="consts", bufs=1) as consts,
            tc.tile_pool(name="acts", bufs=2) as acts,
            tc.tile_pool(name="mpool", bufs=8) as mpool,
            tc.tile_pool(name="spool", bufs=4) as spool,
            tc.tile_pool(name="hsb", bufs=6) as hsb,
            tc.tile_pool(name="psA", bufs=4, space="PSUM") as psA,
            tc.tile_pool(name="psT", bufs=2, space="PSUM") as psT,
            tc.tile_pool(name="psW", bufs=2, space="PSUM") as psW,
        ):
            ident = consts.tile([128, 128], F32)
            make_identity(nc, ident[:])
            iota = consts.tile([128, 128], BF16)
            nc.sync.dma_start(iota[:], iota_in[:])
            dinv = consts.tile([128, T], F32)
            nc.sync.dma_start(dinv[:], dinv_in[:])
            idx_all = consts.tile([128, TOTCOL], I16)
            nc.sync.dma_start(idx_all[:], idx_in[:])
            slot_all = consts.tile([128, TOTBLK], BF16)
            nc.sync.dma_start(slot_all[:], slot_in[:])
            W_sb = []
            for i, (fi, fo) in enumerate(zip(fan_in, WIDTHS)):
                w = consts.tile([fi, fo], BF16, tag=f"W{i}")
                nc.sync.dma_start(w[:], W_in[i][:])
                W_sb.append(w)
            b_sb = []
            for i in range(NL - 1):
                bb = consts.tile([WIDTHS[i], 1], F32, tag=f"b{i}")
                nc.sync.dma_start(bb[:], b_in[i][:, None])
                b_sb.append(bb)
            b5b = consts.tile([128, WIDTHS[NL - 1]], F32)
            nc.sync.dma_start(b5b[:], b5b_in[:])

            act = acts.tile([128, per_pad], BF16, tag="act")
            nc.sync.dma_start(act[:], xT_in[:])

            for l, (fi, fo) in list(enumerate(zip(fan_in, WIDTHS)))[:NL]:
                # ---- transform: H = (act.T @ W) * dinv, quarter by quarter;
                #      AllGather chunk q right after quarter q is written.
                h_own = dram.tile([per_pad, g_el], BF16, tag=f"hown{l}")
                h_gt = [
                    dram.tile([chunk_rows, g_el], BF16, tag=f"hfull{l}_{k}",
                              name=f"h_gt{l}_{k}", addr_space="Shared")
                    for k in range(Q)
                ]
                for t0 in range(0, T, G):
                    hs = hsb.tile([128, G * fo], BF16, tag="hs")
                    for tt in range(G):
                        t = t0 + tt
                        hp = psW.tile([128, fo], F32, tag="psW", space="PSUM")
                        nc.tensor.matmul(
                            out=hp[:],
                            lhsT=act[:fi, t * 128:(t + 1) * 128],
                            rhs=W_sb[l][:],
                            start=True, stop=True,
                        )
                        nc.vector.tensor_scalar(
                            hs[:, tt * fo:(tt + 1) * fo], hp[:],
                            dinv[:, t:t + 1], None,
                            mybir.AluOpType.mult,
                        )
                    dst = (h_own[t0 * 128:(t0 + G) * 128, :fo]
                           .rearrange("(b p) f -> p b f", p=128))
                    nc.sync.dma_start(
                        dst,
                        hs[:].rearrange("p (b f) -> p b f", b=G))
                    if (t0 + G) % TPQ == 0:
                        q = (t0 + G) // TPQ - 1
                        nc.gpsimd.collective_compute(
                            "AllGather",
                            mybir.AluOpType.bypass,
                            replica_groups=[list(range(N_CORES))],
                            ins=[h_own[q * TPQ * 128:(q + 1) * TPQ * 128, :]
                                 .opt()],
                            outs=[h_gt[q][:].opt()],
                        )

                # ---- aggregate, per 5-tile group ----
                if l < NL - 1:
                    next_act = acts.tile([128, per_pad], BF16, tag="act")
                else:
                    next_act = None
                ob = None
                for gi in range(NGRP):
                    Ms = {}
                    for k in range(Q):
                        nbk = NB[gi][k]
                        if nbk == 0:
                            continue
                        M = mpool.tile([128, NBMAX * g_el], BF16, tag="M")
                        nc.gpsimd.dma_gather(
                            out_ap=M[:, :nbk * g_el]
                                .rearrange("p (c f) -> p c f", c=nbk),
                            in_ap=h_gt[k][:],
                            idxs_ap=idx_all[:, col0[gi][k]:
                                            col0[gi][k] + nbk * 8],
                            num_idxs=nbk * 128,
                            num_idxs_reg=nbk * 128,
                            elem_size=g_el,
                            elem_step=g_el,
                            single_packet=False,
                            queue_num=k,
                        )
                        Ms[k] = M
                    for ti in range(G):
                        t = gi * G + ti
                        tot = sum(nb_u[t][k] for k in range(Q))
                        agg = psA.tile([128, fo], F32, tag="agg",
                                       space="PSUM")
                        done = 0
                        for k in range(Q):
                            nbtk = nb_u[t][k]
                            if nbtk == 0:
                                continue
                            S = spool.tile([128, SMAX * 128], BF16, tag="S")
                            gb = blk0[gi][k] + off_u[t][k]
                            nc.vector.tensor_tensor(
                                out=S[:, :nbtk * 128]
                                    .rearrange("p (b j) -> p b j", b=nbtk),
                                in0=slot_all[:, gb:gb + nbtk, None]
                                    .to_broadcast([128, nbtk, 128]),
                                in1=iota[:, None, :]
                                    .to_broadcast([128, nbtk, 128]),
                                op=mybir.AluOpType.is_equal,
                            )
                            for b in range(nbtk):
                                o = (off_u[t][k] + b) * g_el
                                nc.tensor.matmul(
                                    out=agg[:],
                                    lhsT=S[:, b * 128:(b + 1) * 128],
                                    rhs=Ms[k][:, o:o + fo],
                                    start=(done == 0),
                                    stop=(done == tot - 1),
                                )
                                done += 1
                        sc = hsb.tile([128, fo], F32, tag="sc")
                        nc.vector.tensor_scalar(
                            sc[:], agg[:], dinv[:, t:t + 1], None,
                            mybir.AluOpType.mult,
                        )
                        if l < NL - 1:
                            tp = psT.tile([128, 128], F32, tag="psT",
                                          space="PSUM")
                            nc.tensor.transpose(
                                out=tp[:fo, :], in_=sc[:], identity=ident[:])
                            nc.scalar.activation(
                                out=next_act[:fo, t * 128:(t + 1) * 128],
                                in_=tp[:fo, :],
                                func=mybir.ActivationFunctionType.Relu,
                                bias=b_sb[l][:],
                                scale=1.0,
                            )
                        else:
                            sc2 = hsb.tile([128, fo], F32, tag="sc2")
                            nc.vector.tensor_add(sc2[:], sc[:], b5b[:, :fo])
                            if ti == 0:
                                ob = hsb.tile([128, G * fo], F32, tag="ob")
                            nc.scalar.activation(
                                out=ob[:, ti * fo:(ti + 1) * fo],
                                in_=sc2[:],
                                func=mybir.ActivationFunctionType.Relu,
                            )
                            if ti == G - 1:
                                t0 = gi * G
                                nc.sync.dma_start(
                                    out_ext[t0 * 128:(t0 + G) * 128, :]
                                        .rearrange("(b p) f -> p b f", p=128),
                                    ob[:].rearrange("p (b f) -> p b f", b=G))
                act = next_act
    nc.compile()
    return nc


# ------------------------------------------------------------------ host prep


def _preprocess(x, edge_index):
    N = x.shape[0]
    per = N // N_CORES
    per_pad = T * 128
    qrows = TPQ * 128
    chunk_rows = N_CORES * qrows
    HN = N_CORES * per_pad
    assert chunk_rows <= 32768

    src = np.concatenate([edge_index[0], np.arange(N)]).astype(np.int64)
    dst = np.concatenate([edge_index[1], np.arange(N)]).astype(np.int64)
    deg = np.bincount(dst, minlength=N).astype(np.float32)
    dinv = (1.0 / np.sqrt(deg)).astype(np.float32)

    node_core = (np.arange(N) // per).astype(np.int32)

    # per-core LPT assignment of nodes to tiles, balancing in-degree
    tile_of = np.empty(N, np.int32)
    slot_of = np.empty(N, np.int32)
    for c in range(N_CORES):
        nodes = np.arange(c * per, (c + 1) * per)
        d = deg[nodes]
        order = np.argsort(-d, kind="stable")
        heap = [(0.0, t) for t in range(T)]
        heapq.heapify(heap)
        counts = np.zeros(T, np.int32)
        tl = np.empty(per, np.int32)
        sl = np.empty(per, np.int32)
        for i in order:
            while True:
                load, t = heapq.heappop(heap)
                if counts[t] < 128:
                    break
            tl[i] = t
            sl[i] = counts[t]
            counts[t] += 1
            heapq.heappush(heap, (load + float(d[i]), t))
        tile_of[nodes] = tl
        slot_of[nodes] = sl

    tilepos = tile_of * 128 + slot_of
    qq = tilepos // qrows
    rr = tilepos % qrows
    loc_in_chunk = node_core.astype(np.int64) * qrows + rr  # < chunk_rows

    e_core = node_core[dst]
    e_tile = tile_of[dst]
    e_chunk = qq[src]
    e_loc = loc_in_chunk[src].astype(np.int16)
    key = (e_core.astype(np.int64) * T + e_tile) * Q + e_chunk
    cnt = np.bincount(key, minlength=N_CORES * T * Q) \
        .reshape(N_CORES, T, Q)
    nb_u = np.ceil(cnt.max(axis=0) / 128).astype(np.int64)  # [T, Q]

    # op structure: op (g, k) covers tiles [g*G, (g+1)*G) of chunk k
    NB = np.zeros((NGRP, Q), np.int64)
    off_u = np.zeros((T, Q), np.int64)
    for g in range(NGRP):
        for k in range(Q):
            c0 = 0
            for ti in range(G):
                t = g * G + ti
                off_u[t, k] = c0
                c0 += nb_u[t, k]
            NB[g, k] = c0
    col0 = np.zeros((NGRP, Q), np.int64)
    blk0 = np.zeros((NGRP, Q), np.int64)
    acc_c = 0
    acc_b = 0
    for g in range(NGRP):
        for k in range(Q):
            col0[g, k] = acc_c
            blk0[g, k] = acc_b
            acc_c += NB[g, k] * 8
            acc_b += NB[g, k]
    TOTCOL = int(acc_c)
    TOTBLK = int(acc_b)
    NBMAX = int(NB.max())
    SMAX = int(nb_u.max())

    eorder = np.argsort(key, kind="stable")
    starts = np.zeros(N_CORES * T * Q + 1, np.int64)
    starts[1:] = np.cumsum(cnt.reshape(-1))
    pos = np.arange(len(key)) - starts[key[eorder]]

    sc_, st_, sk_ = e_core[eorder], e_tile[eorder], e_chunk[eorder]
    sg_ = st_ // G
    i_elem = (off_u[st_, sk_] + pos // 128) * 128 + pos % 128
    colpos = col0[sg_, sk_] + i_elem // 16
    rowpos = i_elem % 16

    idx16 = np.zeros((N_CORES, 16, TOTCOL), np.int16)
    idx16[sc_, rowpos, colpos] = e_loc[eorder]
    idx_arr = np.ascontiguousarray(np.tile(idx16, (1, 8, 1)))

    gb_ = blk0[sg_, sk_] + off_u[st_, sk_] + pos // 128
    slot_arr = np.full((N_CORES, 128, TOTBLK), 255.0, ml_dtypes.bfloat16)
    slot_arr[sc_, pos % 128, gb_] = slot_of[dst[eorder]] \
        .astype(ml_dtypes.bfloat16)

    xt = np.zeros((N_CORES, per_pad, IN_C), np.float32)
    xt[node_core, tilepos] = x
    xt = np.ascontiguousarray(
        xt.transpose(0, 2, 1)).astype(ml_dtypes.bfloat16)

    dv = np.ones((N_CORES, T, 128), np.float32)
    dv[node_core, tile_of, slot_of] = dinv
    dv = np.ascontiguousarray(dv.transpose(0, 2, 1))

    struct = dict(
        per_pad=per_pad, chunk_rows=chunk_rows, HN=HN,
        NB=tuple(map(tuple, NB)),
        nb_u=tuple(map(tuple, nb_u)),
        off_u=tuple(map(tuple, off_u)),
        col0=tuple(map(tuple, col0)),
        blk0=tuple(map(tuple, blk0)),
        TOTCOL=TOTCOL, TOTBLK=TOTBLK, NBMAX=NBMAX, SMAX=SMAX,
    )
    return dict(
        N=N, struct=struct, idx=idx_arr, slot=slot_arr, xt=xt, dv=dv,
        node_core=node_core, tilepos=tilepos,
    )


_PROGRAM_CACHE = {}


def _struct_key(struct):
    return (struct["per_pad"], struct["chunk_rows"], struct["NB"],
            struct["nb_u"], struct["TOTCOL"])


def make_in_maps(p, inputs):
    Ws = [np.ascontiguousarray(
        np.asarray(inputs[f"W{i + 1}"], np.float32)
        .astype(ml_dtypes.bfloat16)) for i in range(5)]
    bs = [np.ascontiguousarray(np.asarray(inputs[f"b{i + 1}"], np.float32))
          for i in range(5)]
    iota = np.ascontiguousarray(
        np.tile(np.arange(128, dtype=ml_dtypes.bfloat16), (128, 1)))
    b5b = np.ascontiguousarray(np.tile(bs[NL - 1], (128, 1)))
    in_maps = []
    for c in range(N_CORES):
        m = {
            "xT": p["xt"][c],
            "idx": p["idx"][c],
            "slot": p["slot"][c],
            "dinv": p["dv"][c],
            "iota": iota,
            "b5b": b5b,
        }
        for i in range(5):
            m[f"W{i + 1}"] = Ws[i]
        for i in range(NL - 1):
            m[f"b{i + 1}"] = bs[i]
        in_maps.append(m)
    return in_maps


def kernel(**inputs):
    x = np.ascontiguousarray(np.asarray(inputs["x"], dtype=np.float32))
    edge_index = np.asarray(inputs["edge_index"])

    p = _preprocess(x, edge_index)

    key = _struct_key(p["struct"])
    if key not in _PROGRAM_CACHE:
        _PROGRAM_CACHE[key] = _build_program(p["struct"])
    nc = _PROGRAM_CACHE[key]

    in_maps = make_in_maps(p, inputs)
    res = run_bass_kernel_spmd(nc, in_maps, core_ids=list(range(N_CORES)))

    shards = np.stack([res.results[c]["out"] for c in range(N_CORES)])
    out = np.empty((p["N"], WIDTHS[NL - 1]), np.float32)
    out[:] = shards[p["node_core"], p["tilepos"]]
    return out


# revision 8
# speedup vs baseline: 1.7011x; 1.1452x over previous
"""Trainium2 Bass kernel for nn_DownConvLayers (5-layer GCN, N=100k, E=1.6M).

Strategy (8 NeuronCores, SPMD), v3:
  - Shard destination nodes across 8 cores; T=100 tiles x 128 slots per core
    (LPT-balanced by in-degree).  Norm factorizes: fold dinv[src] into the
    feature table rows, dinv[dst] into the epilogue.
  - Table rows quarter-interleaved: chunk k holds quarter k of every core's
    rows, so the per-layer AllGather splits into 4 Shared-output
    sub-collectives that pipeline with gathers of earlier chunks.
  - ONE dma_gather per (5-tile group, chunk): 80 ops/layer.  The SWDGE
    descriptor generation (~2.4ns/desc on HW) on the single Pool engine is
    the roofline; everything else is arranged to hide under it:
      * transform of layer l+1 is emitted per-group INSIDE layer l's
        aggregation loop (right after the epilogue writes those act cols),
        and the 4 AllGathers of l+1 fire right after the layer's gathers -
        so Pool rolls from layer l gathers into layer l+1 gathers with only
        the first sub-AllGather's latency as a bubble;
      * the first two groups' gathers are issued chunk-interleaved so the
        first gather needs only sub-AllGather 0.
  - Self-loop messages are dropped from the edge list; their contribution
    dinv^3 * (act_tile @ W) is recomputed on the (idle) PE at epilogue time.
  - idx/slot arrays are layer-invariant and SBUF-resident.
  - Scatter matrices S built just-in-time per (tile,chunk) on DVE;
    segment-sum psum[128 dst, fo] += S_b^T @ M_b on PE; epilogue scales by
    dinv[dst], PE-transposes, relu(x+b) on ACT into the next act buffer.
"""

import heapq
import math

import ml_dtypes
import numpy as np

import concourse.bacc as bacc
import concourse.tile as tile
from concourse import bass, mybir
from concourse.bass_utils import run_bass_kernel_spmd
from concourse.masks import make_identity

N_CORES = 8
IN_C = 128
WIDTHS = [128, 64, 32, 16, 8]
F32 = mybir.dt.float32
BF16 = mybir.dt.bfloat16
I16 = mybir.dt.int16
NQ = 4          # SWDGE queues
NL = 5          # layers to build (debug knob)
T = 100         # tiles per core
TPQ = 25        # tiles per quarter
Q = 4           # quarters == chunks
G = 5           # tiles per gather group
NGRP = T // G


# ---------------------------------------------------------------- device code


def _build_program(struct):
    per_pad = struct["per_pad"]
    chunk_rows = struct["chunk_rows"]
    NB = struct["NB"]          # [NGRP][Q] blocks per op
    nb_u = struct["nb_u"]      # [T][Q] blocks per (tile, chunk)
    off_u = struct["off_u"]    # [T][Q] block offset of tile within its op
    col0 = struct["col0"]      # [NGRP][Q] idx col offset of op
    blk0 = struct["blk0"]      # [NGRP][Q] global block offset of op
    TOTCOL = struct["TOTCOL"]
    TOTBLK = struct["TOTBLK"]
    NBMAX = struct["NBMAX"]
    SMAX = struct["SMAX"]
    g_el = 128                 # gather element: 128 bf16 cols = 256B

    fan_in = [IN_C] + WIDTHS[:-1]
    nc = bacc.Bacc("TRN2", target_bir_lowering=False, debug=False,
                   num_devices=N_CORES, num_swdge_queues=NQ)

    xT_in = nc.dram_tensor("xT", [128, per_pad], BF16, kind="ExternalInput")
    idx_in = nc.dram_tensor("idx", [128, TOTCOL], I16, kind="ExternalInput")
    slot_in = nc.dram_tensor("slot", [128, TOTBLK], BF16,
                             kind="ExternalInput")
    dinv_in = nc.dram_tensor("dinv", [128, T], F32, kind="ExternalInput")
    dinv3_in = nc.dram_tensor("dinv3", [128, T], F32, kind="ExternalInput")
    iota_in = nc.dram_tensor("iota", [128, 128], BF16, kind="ExternalInput")
    W_in = [
        nc.dram_tensor(f"W{i + 1}", [fi, fo], BF16, kind="ExternalInput")
        for i, (fi, fo) in enumerate(zip(fan_in, WIDTHS))
    ]
    b_in = [
        nc.dram_tensor(f"b{i + 1}", [WIDTHS[i]], F32, kind="ExternalInput")
        for i in range(max(NL - 1, 0))
    ]
    b5b_in = nc.dram_tensor("b5b", [128, WIDTHS[NL - 1]], F32,
                            kind="ExternalInput")
    out_ext = nc.dram_tensor("out", [per_pad, WIDTHS[NL - 1]], F32,
                             kind="ExternalOutput")

    with tile.TileContext(nc) as tc:
        with (
            tc.tile_pool(name="dram", bufs=1, space="DRAM") as dram,
            tc.tile_pool(name="consts", bufs=1) as consts,
            tc.tile_pool(name="acts", bufs=2) as acts,
            tc.tile_pool(name="mpool", bufs=8) as mpool,
            tc.tile_pool(name="spool", bufs=4) as spool,
            tc.tile_pool(name="hsb", bufs=6) as hsb,
            tc.tile_pool(name="psA", bufs=3, space="PSUM") as psA,
            tc.tile_pool(name="psT", bufs=2, space="PSUM") as psT,
            tc.tile_pool(name="psW", bufs=3, space="PSUM") as psW,
        ):
            ident = consts.tile([128, 128], F32)
            make_identity(nc, ident[:])
            iota = consts.tile([128, 128], BF16)
            nc.sync.dma_start(iota[:], iota_in[:])
            dinv = consts.tile([128, T], F32)
            nc.sync.dma_start(dinv[:], dinv_in[:])
            dinv3 = consts.tile([128, T], F32)
            nc.sync.dma_start(dinv3[:], dinv3_in[:])
            idx_all = consts.tile([128, TOTCOL], I16)
            nc.sync.dma_start(idx_all[:], idx_in[:])
            slot_all = consts.tile([128, TOTBLK], BF16)
            nc.sync.dma_start(slot_all[:], slot_in[:])
            W_sb = []
            for i, (fi, fo) in enumerate(zip(fan_in, WIDTHS)):
                w = consts.tile([fi, fo], BF16, tag=f"W{i}")
                nc.sync.dma_start(w[:], W_in[i][:])
                W_sb.append(w)
            b_sb = []
            for i in range(NL - 1):
                bb = consts.tile([WIDTHS[i], 1], F32, tag=f"b{i}")
                nc.sync.dma_start(bb[:], b_in[i][:, None])
                b_sb.append(bb)
            b5b = consts.tile([128, WIDTHS[NL - 1]], F32)
            nc.sync.dma_start(b5b[:], b5b_in[:])

            act = acts.tile([128, per_pad], BF16, tag="act")
            nc.sync.dma_start(act[:], xT_in[:])

            h_own = [None] * NL
            h_gt = [None] * NL

            def transform_batch(l, src_act, t0):
                """h(l) for tiles [t0, t0+G) from src_act; AllGather a chunk
                when its quarter completes."""
                fi_l, fo_l = fan_in[l], WIDTHS[l]
                hs = hsb.tile([128, G * 128], BF16, tag="hs")
                for tt in range(G):
                    t = t0 + tt
                    hp = psW.tile([128, fo_l], F32, tag="psW", space="PSUM")
                    nc.tensor.matmul(
                        out=hp[:],
                        lhsT=src_act[:fi_l, t * 128:(t + 1) * 128],
                        rhs=W_sb[l][:],
                        start=True, stop=True,
                    )
                    nc.vector.tensor_scalar(
                        hs[:, tt * fo_l:(tt + 1) * fo_l], hp[:],
                        dinv[:, t:t + 1], None,
                        mybir.AluOpType.mult,
                    )
                dst = (h_own[l][t0 * 128:(t0 + G) * 128, :fo_l]
                       .rearrange("(b p) f -> p b f", p=128))
                nc.sync.dma_start(
                    dst,
                    hs[:, :G * fo_l].rearrange("p (b f) -> p b f", b=G))

            def allgather_chunk(l, q):
                nc.gpsimd.collective_compute(
                    "AllGather",
                    mybir.AluOpType.bypass,
                    replica_groups=[list(range(N_CORES))],
                    ins=[h_own[l][q * TPQ * 128:(q + 1) * TPQ * 128, :]
                         .opt()],
                    outs=[h_gt[l][q][:].opt()],
                )

            def alloc_h(l):
                h_own[l] = dram.tile([per_pad, g_el], BF16, tag=f"hown{l}",
                                     name=f"h_own{l}")
                h_gt[l] = [
                    dram.tile([chunk_rows, g_el], BF16, tag=f"hfull{l}_{k}",
                              name=f"h_gt{l}_{k}", addr_space="Shared")
                    for k in range(Q)
                ]

            # layer-0 transform, standalone
            alloc_h(0)
            for t0 in range(0, T, G):
                transform_batch(0, act, t0)
                if (t0 + G) % TPQ == 0:
                    allgather_chunk(0, (t0 + G) // TPQ - 1)

            for l, (fi, fo) in list(enumerate(zip(fan_in, WIDTHS)))[:NL]:
                if l < NL - 1:
                    next_act = acts.tile([128, per_pad], BF16, tag="act")
                    alloc_h(l + 1)
                else:
                    next_act = None

                # gather issue order: interleave groups 0,1 chunk-by-chunk
                # (first gather only needs sub-AllGather 0), then g2..g19.
                issue = [(g, k) for k in range(Q) for g in (0, 1)] + \
                        [(g, k) for g in range(2, NGRP) for k in range(Q)]
                Ms = {}
                issued = iter(issue)

                def issue_gathers(upto_g):
                    for g, k in issued:
                        nbk = NB[g][k]
                        if nbk > 0:
                            M = mpool.tile([128, NBMAX * g_el], BF16,
                                           tag="M", name="M")
                            nc.gpsimd.dma_gather(
                                out_ap=M[:, :nbk * g_el]
                                    .rearrange("p (c f) -> p c f", c=nbk),
                                in_ap=h_gt[l][k][:],
                                idxs_ap=idx_all[:, col0[g][k]:
                                                col0[g][k] + nbk * 8],
                                num_idxs=nbk * 128,
                                num_idxs_reg=nbk * 128,
                                elem_size=g_el,
                                elem_step=g_el,
                                single_packet=False,
                                queue_num=k,
                            )
                            Ms[(g, k)] = M
                        if g == upto_g and k == Q - 1:
                            return

                ob = None
                for gi in range(NGRP):
                    issue_gathers(gi)
                    for ti in range(G):
                        t = gi * G + ti
                        tot = sum(nb_u[t][k] for k in range(Q))
                        agg = psA.tile([128, fo], F32, tag="agg",
                                       space="PSUM")
                        done = 0
                        for k in range(Q):
                            nbtk = nb_u[t][k]
                            if nbtk == 0:
                                continue
                            S = spool.tile([128, SMAX * 128], BF16, tag="S")
                            gb = blk0[gi][k] + off_u[t][k]
                            nc.vector.tensor_tensor(
                                out=S[:, :nbtk * 128]
                                    .rearrange("p (b j) -> p b j", b=nbtk),
                                in0=slot_all[:, gb:gb + nbtk, None]
                                    .to_broadcast([128, nbtk, 128]),
                                in1=iota[:, None, :]
                                    .to_broadcast([128, nbtk, 128]),
                                op=mybir.AluOpType.is_equal,
                            )
                            for b in range(nbtk):
                                o = (off_u[t][k] + b) * g_el
                                nc.tensor.matmul(
                                    out=agg[:],
                                    lhsT=S[:, b * 128:(b + 1) * 128],
                                    rhs=Ms[(gi, k)][:, o:o + fo],
                                    start=(done == 0),
                                    stop=(done == tot - 1),
                                )
                                done += 1
                        # self-loop contribution: dinv^3 * (act_t @ W)
                        hp = psW.tile([128, fo], F32, tag="psW",
                                      space="PSUM")
                        nc.tensor.matmul(
                            out=hp[:],
                            lhsT=act[:fi, t * 128:(t + 1) * 128],
                            rhs=W_sb[l][:],
                            start=True, stop=True,
                        )
                        sc = hsb.tile([128, fo], F32, tag="sc")
                        nc.vector.tensor_scalar(
                            sc[:], agg[:], dinv[:, t:t + 1], None,
                            mybir.AluOpType.mult,
                        )
                        t3 = hsb.tile([128, fo], F32, tag="t3")
                        nc.vector.tensor_scalar(
                            t3[:], hp[:], dinv3[:, t:t + 1], None,
                            mybir.AluOpType.mult,
                        )
                        sc2 = hsb.tile([128, fo], F32, tag="sc2")
                        nc.vector.tensor_add(sc2[:], sc[:], t3[:])
                        if l < NL - 1:
                            tp = psT.tile([128, 128], F32, tag="psT",
                                          space="PSUM")
                            nc.tensor.transpose(
                                out=tp[:fo, :], in_=sc2[:],
                                identity=ident[:])
                            nc.scalar.activation(
                                out=next_act[:fo, t * 128:(t + 1) * 128],
                                in_=tp[:fo, :],
                                func=mybir.ActivationFunctionType.Relu,
                                bias=b_sb[l][:],
                                scale=1.0,
                            )
                        else:
                            sc3 = hsb.tile([128, fo], F32, tag="sc3")
                            nc.vector.tensor_add(sc3[:], sc2[:],
                                                 b5b[:, :fo])
                            if ti == 0:
                                ob = hsb.tile([128, G * fo], F32, tag="ob")
                            nc.scalar.activation(
                                out=ob[:, ti * fo:(ti + 1) * fo],
                                in_=sc3[:],
                                func=mybir.ActivationFunctionType.Relu,
                            )
                            if ti == G - 1:
                                t0b = gi * G
                                nc.sync.dma_start(
                                    out_ext[t0b * 128:(t0b + G) * 128, :]
                                        .rearrange("(b p) f -> p b f",
                                                   p=128),
                                    ob[:].rearrange("p (b f) -> p b f",
                                                    b=G))
                    # transform of layer l+1 for this group's tiles
                    if l < NL - 1:
                        transform_batch(l + 1, next_act, gi * G)
                # fire the next layer's sub-AllGathers after this layer's
                # gathers so Pool doesn't stall mid-layer
                if l < NL - 1:
                    for q in range(Q):
                        allgather_chunk(l + 1, q)
                act = next_act
    nc.compile()
    return nc


# ------------------------------------------------------------------ host prep


def _preprocess(x, edge_index):
    N = x.shape[0]
    per = N // N_CORES
    per_pad = T * 128
    qrows = TPQ * 128
    chunk_rows = N_CORES * qrows
    HN = N_CORES * per_pad
    assert chunk_rows <= 32768

    src = np.asarray(edge_index[0], np.int64)
    dst = np.asarray(edge_index[1], np.int64)
    deg = (np.bincount(dst, minlength=N) + 1).astype(np.float32)
    dinv = (1.0 / np.sqrt(deg)).astype(np.float32)

    node_core = (np.arange(N) // per).astype(np.int32)

    # per-core LPT assignment of nodes to tiles, balancing in-degree
    tile_of = np.empty(N, np.int32)
    slot_of = np.empty(N, np.int32)
    for c in range(N_CORES):
        nodes = np.arange(c * per, (c + 1) * per)
        d = deg[nodes]
        order = np.argsort(-d, kind="stable")
        heap = [(0.0, t) for t in range(T)]
        heapq.heapify(heap)
        counts = np.zeros(T, np.int32)
        tl = np.empty(per, np.int32)
        sl = np.empty(per, np.int32)
        for i in order:
            while True:
                load, t = heapq.heappop(heap)
                if counts[t] < 128:
                    break
            tl[i] = t
            sl[i] = counts[t]
            counts[t] += 1
            heapq.heappush(heap, (load + float(d[i]), t))
        tile_of[nodes] = tl
        slot_of[nodes] = sl

    tilepos = tile_of * 128 + slot_of
    qq = tilepos // qrows
    rr = tilepos % qrows
    loc_in_chunk = node_core.astype(np.int64) * qrows + rr  # < chunk_rows

    e_core = node_core[dst]
    e_tile = tile_of[dst]
    e_chunk = qq[src]
    e_loc = loc_in_chunk[src].astype(np.int16)
    key = (e_core.astype(np.int64) * T + e_tile) * Q + e_chunk
    cnt = np.bincount(key, minlength=N_CORES * T * Q) \
        .reshape(N_CORES, T, Q)
    nb_u = np.ceil(cnt.max(axis=0) / 128).astype(np.int64)  # [T, Q]

    # op structure: op (g, k) covers tiles [g*G, (g+1)*G) of chunk k
    NB = np.zeros((NGRP, Q), np.int64)
    off_u = np.zeros((T, Q), np.int64)
    for g in range(NGRP):
        for k in range(Q):
            c0 = 0
            for ti in range(G):
                t = g * G + ti
                off_u[t, k] = c0
                c0 += nb_u[t, k]
            NB[g, k] = c0
    col0 = np.zeros((NGRP, Q), np.int64)
    blk0 = np.zeros((NGRP, Q), np.int64)
    acc_c = 0
    acc_b = 0
    for g in range(NGRP):
        for k in range(Q):
            col0[g, k] = acc_c
            blk0[g, k] = acc_b
            acc_c += NB[g, k] * 8
            acc_b += NB[g, k]
    TOTCOL = int(acc_c)
    TOTBLK = int(acc_b)
    NBMAX = int(NB.max())
    SMAX = int(nb_u.max())

    eorder = np.argsort(key, kind="stable")
    starts = np.zeros(N_CORES * T * Q + 1, np.int64)
    starts[1:] = np.cumsum(cnt.reshape(-1))
    pos = np.arange(len(key)) - starts[key[eorder]]

    sc_, st_, sk_ = e_core[eorder], e_tile[eorder], e_chunk[eorder]
    sg_ = st_ // G
    i_elem = (off_u[st_, sk_] + pos // 128) * 128 + pos % 128
    colpos = col0[sg_, sk_] + i_elem // 16
    rowpos = i_elem % 16

    idx16 = np.zeros((N_CORES, 16, TOTCOL), np.int16)
    idx16[sc_, rowpos, colpos] = e_loc[eorder]
    idx_arr = np.ascontiguousarray(np.tile(idx16, (1, 8, 1)))

    gb_ = blk0[sg_, sk_] + off_u[st_, sk_] + pos // 128
    slot_arr = np.full((N_CORES, 128, TOTBLK), 255.0, ml_dtypes.bfloat16)
    slot_arr[sc_, pos % 128, gb_] = slot_of[dst[eorder]] \
        .astype(ml_dtypes.bfloat16)

    xt = np.zeros((N_CORES, per_pad, IN_C), np.float32)
    xt[node_core, tilepos] = x
    xt = np.ascontiguousarray(
        xt.transpose(0, 2, 1)).astype(ml_dtypes.bfloat16)

    dv = np.ones((N_CORES, T, 128), np.float32)
    dv[node_core, tile_of, slot_of] = dinv
    dv = np.ascontiguousarray(dv.transpose(0, 2, 1))
    # self-loop term is added after the final dinv[dst] scale, so it needs
    # dinv[dst]^2 (norm of a self edge), not ^3
    dv3 = np.ascontiguousarray(dv ** 2)

    struct = dict(
        per_pad=per_pad, chunk_rows=chunk_rows, HN=HN,
        NB=tuple(map(tuple, NB)),
        nb_u=tuple(map(tuple, nb_u)),
        off_u=tuple(map(tuple, off_u)),
        col0=tuple(map(tuple, col0)),
        blk0=tuple(map(tuple, blk0)),
        TOTCOL=TOTCOL, TOTBLK=TOTBLK, NBMAX=NBMAX, SMAX=SMAX,
    )
    return dict(
        N=N, struct=struct, idx=idx_arr, slot=slot_arr, xt=xt, dv=dv,
        dv3=dv3, node_core=node_core, tilepos=tilepos,
    )


_PROGRAM_CACHE = {}


def _struct_key(struct):
    return (struct["per_pad"], struct["chunk_rows"], struct["NB"],
            struct["nb_u"], struct["TOTCOL"])


def make_in_maps(p, inputs):
    Ws = [np.ascontiguousarray(
        np.asarray(inputs[f"W{i + 1}"], np.float32)
        .astype(ml_dtypes.bfloat16)) for i in range(5)]
    bs = [np.ascontiguousarray(np.asarray(inputs[f"b{i + 1}"], np.float32))
          for i in range(5)]
    iota = np.ascontiguousarray(
        np.tile(np.arange(128, dtype=ml_dtypes.bfloat16), (128, 1)))
    b5b = np.ascontiguousarray(np.tile(bs[NL - 1], (128, 1)))
    in_maps = []
    for c in range(N_CORES):
        m = {
            "xT": p["xt"][c],
            "idx": p["idx"][c],
            "slot": p["slot"][c],
            "dinv": p["dv"][c],
            "dinv3": p["dv3"][c],
            "iota": iota,
            "b5b": b5b,
        }
        for i in range(5):
            m[f"W{i + 1}"] = Ws[i]
        for i in range(NL - 1):
            m[f"b{i + 1}"] = bs[i]
        in_maps.append(m)
    return in_maps


def kernel(**inputs):
    x = np.ascontiguousarray(np.asarray(inputs["x"], dtype=np.float32))
    edge_index = np.asarray(inputs["edge_index"])

    p = _preprocess(x, edge_index)

    key = _struct_key(p["struct"])
    if key not in _PROGRAM_CACHE:
        _PROGRAM_CACHE[key] = _build_program(p["struct"])
    nc = _PROGRAM_CACHE[key]

    in_maps = make_in_maps(p, inputs)
    res = run_bass_kernel_spmd(nc, in_maps, core_ids=list(range(N_CORES)))

    shards = np.stack([res.results[c]["out"] for c in range(N_CORES)])
    out = np.empty((p["N"], WIDTHS[NL - 1]), np.float32)
    out[:] = shards[p["node_core"], p["tilepos"]]
    return out
